# revision 3
# baseline (speedup 1.0000x reference)
"""AgentAttention Trainium2 kernel (B=64, N=1024, C=512, M=16 agents) on 8 NeuronCores.

Data-parallel over batch: each core processes 8 batch elements. No collectives.

Math (per batch element, reference semantics):
    Q = x@Wq.T+bq ; K = x@Wk.T+bk ; V = x@Wv.T+bv
    A = group-mean of Q over 64-token groups          -> [16, C]
    S1 = softmax(Q A^T / sqrt(C), axis=agents)        -> [N, 16]
    S2 = softmax(A K^T / sqrt(C), axis=tokens)        -> [16, N]
    out = (S1 @ (S2 @ V)) @ Wo.T + bo

Algebraic restructuring (exact in real arithmetic):
    - bv never materialized: softmax rows sum to 1 =>  out += (Wo@bv + bo) == b'
    - A uses group-SUM; the 1/64 is folded into the logit scale s = 1/(64*sqrt(C))
    - re-association: S1@((S2@x)@(Wv^T Wo^T)) replaces the O(N*C^2) V- and
      output-projections with agent-space (M=16) ops; Wvo^T = (Wo@Wv)^T is
      precomputed on host in float64.
    - the A@bk logit shift is constant along the stage-2 softmax axis and
      cancels; it is dropped entirely.
    - stage-1 softmax normalizer r1 applied as a per-row scale on the final
      output tile; stage-2 normalizer r2 applied when evicting (S2@x) from PSUM.

Perf structure (v3):
    - Tensor engine is the bottleneck; everything else is arranged to keep it
      densely fed and HAM-warm:
        * ~10 dummy warm-up matmuls at t=0 (overlap the initial DMA) so the
          HAM clock gate opens before real work arrives.
        * no PE-mode transposes: exr -> exrT via DMA transpose (XBAR).
        * 48-row logit block (E1 rows 0:16, E2 rows 32:48; 32-aligned bases).
        * software pipeline ordered so every PE group's inputs are ready a
          full round ahead: round r = l12(r), afw(r-2), ex(r-1), outp(r-2).
    - bulk HBM loads/stores issued from GpSimd (SWDGE); the Sync HWDGE ring
      only carries DMA transposes (avoids xbar-mode transition serialization).
    - PSUM budget exactly 8 banks: 2 logit + 2 (ex/afw) + 4 outp.
    - all HBM<->SBUF transfers host-permuted so every SBUF partition
      reads/writes a single contiguous block (large DMA packets).
"""

import sys

if "/opt/trn_rl_repo" not in sys.path:
    sys.path.insert(0, "/opt/trn_rl_repo")

import numpy as np
import ml_dtypes

import concourse.bass as bass
import concourse.mybir as mybir
import concourse.tile as tile
from concourse import bacc
from concourse.bass import ts, ds
from concourse.bass_utils import run_bass_kernel_spmd

BF16 = mybir.dt.bfloat16
F32 = mybir.dt.float32
F8 = mybir.dt.float8e4

N_CORES = 8
B = 64
B_LOC = B // N_CORES  # 8 batches per core
N = 1024              # tokens
C = 512               # channels
M = 16                # agents
G = N // M            # 64-token pooling groups
P = 128
ND = C // P           # 4 channel chunks
NN = N // P           # 8 token chunks of 128
SCALE = 1.0 / (G * np.sqrt(C))  # logit scale (1/64 pooling fold included)
NWARM = 10            # PE warm-up matmuls

# test harness may override (e.g. {"trace": True, "tmpdir": ...})
_RUN_KWARGS = {}
_LAST_RESULTS = None


def _build_program():
    nc = bacc.Bacc("TRN2", target_bir_lowering=False, debug=False,
                   num_devices=N_CORES)

    xt_d = nc.dram_tensor("xt", [B_LOC, P, 2, 2, N], F8, kind="ExternalInput")
    xn_d = nc.dram_tensor("xn", [B_LOC, P, NN, C], BF16, kind="ExternalInput")
    wvo_d = nc.dram_tensor("wvoT", [P, ND, C], BF16, kind="ExternalInput")
    bp_d = nc.dram_tensor("bp", [M, C], F32, kind="ExternalInput")
    awqk_d = nc.dram_tensor("awqk", [P, 2, 2, B_LOC, 3 * M], F8,
                            kind="ExternalInput")
    c12b_d = nc.dram_tensor("c12b", [3 * M, B_LOC], F32, kind="ExternalInput")
    out_d = nc.dram_tensor("out", [B_LOC, P, NN, C], BF16, kind="ExternalOutput")

    with tile.TileContext(nc) as tc:
        with (
            tc.tile_pool(name="const", bufs=1) as const,
            tc.tile_pool(name="pxt", bufs=4) as pxt,
            tc.tile_pool(name="pxn", bufs=5) as pxn,
            tc.tile_pool(name="pe12", bufs=4) as pe12,
            tc.tile_pool(name="pe12p", bufs=3) as pe12p,
            tc.tile_pool(name="psmall", bufs=6) as psmall,
            tc.tile_pool(name="pout", bufs=3) as pout,
            tc.tile_pool(name="ps_out", bufs=4, space="PSUM") as ps_out,
            tc.tile_pool(name="ps_log", bufs=2, space="PSUM") as ps_log,
            tc.tile_pool(name="ps_se", bufs=2, space="PSUM") as ps_se,
        ):
            wvo_s = const.tile([P, ND, C], BF16)
            bp_s = const.tile([M, C], F32)
            awqk_s = const.tile([P, 2, 2, B_LOC, 3 * M], F8)
            c12b_s = const.tile([3 * M, B_LOC], F32)
            warm = const.tile([P, 512 + P], BF16)

            # small consts go first on the (otherwise transpose-only) sync ring
            nc.sync.dma_start(awqk_s[:], awqk_d.ap())
            nc.sync.dma_start(c12b_s[:], c12b_d.ap())
            nc.sync.dma_start(bp_s[:], bp_d.ap())
            # preload the ACT exp table before the first real activation
            scr = const.tile([1, 2], F32)
            nc.vector.memset(scr[:], 0.0)
            nc.scalar.activation(scr[:], scr[:],
                                 mybir.ActivationFunctionType.Exp)
            # PE warm-up: dense dummy matmuls while the first DMAs land.
            # Opens the HAM clock gate (1.2 -> 2.4 GHz) before real work.
            nc.vector.memset(warm[:], 0.0)
            for w in range(NWARM):
                pw = ps_out.tile([P, 512], F32, tag="mm", name=f"warm_{w}")
                nc.tensor.matmul(pw[:], warm[:, 512:], warm[:, 0:512],
                                 start=True, stop=True)

            st = [dict() for _ in range(B_LOC)]

            def load_xt(b):
                s = st[b]
                s["xt"] = xt = pxt.tile([P, 2, 2, N], F8, tag="xt", name=f"xt_{b}")
                nc.gpsimd.dma_start(xt[:], xt_d.ap()[b])

            def load_xn(b):
                s = st[b]
                s["xn"] = xn = pxn.tile([P, NN, C], BF16, tag="xn", name=f"xn_{b}")
                nc.gpsimd.dma_start(xn[:], xn_d.ap()[b])

            def l12(b):
                # e12t rows 0:16 = E1 = exp(s*Q A^T + c1); rows 32:48 = E2 = exp(s*A K^T)
                # (rows 16:32 pad: engines need partition bases at multiples of 32)
                s = st[b]
                s["e12t"] = e12t = pe12.tile([3 * M, N], BF16, tag="e12t",
                                             name=f"e12t_{b}")
                d2 = psmall.tile([3 * M, 2], F32, tag="d2", name=f"d2_{b}")
                s["e12p"] = e12p = pe12p.tile([P, NN, 3 * M], BF16, tag="e12p",
                                              name=f"e12p_{b}")
                h = NN // 2
                for ni in range(2):
                    lg = ps_log.tile([3 * M, 512], F32, tag="log", name=f"log_{b}")
                    for kk in range(2):
                        nc.tensor.matmul(
                            lg[:], awqk_s[:, kk, :, b, :],
                            s["xt"][:, kk, :, ts(ni, 512)],
                            start=(kk == 0), stop=(kk == 1),
                            perf_mode=mybir.MatmulPerfMode.DoubleRow)
                    nc.scalar.activation(
                        e12t[:, ts(ni, 512)], lg[:],
                        mybir.ActivationFunctionType.Exp,
                        bias=c12b_s[:, b:b + 1], scale=float(SCALE),
                        accum_out=d2[:, ni:ni + 1])
                    nc.sync.dma_start_transpose(
                        e12p[:, ni * h:(ni + 1) * h, :], e12t[:, ts(ni, 512)])
                s["d2"] = d2

            def r2c(b):
                s = st[b]
                d2 = s["d2"]
                d2s = psmall.tile([M, 1], F32, tag="d2s", name=f"d2s_{b}")
                nc.vector.tensor_add(d2s[:], d2[2 * M:3 * M, 0:1], d2[2 * M:3 * M, 1:2])
                s["r2"] = r2 = psmall.tile([M, 1], F32, tag="r2", name=f"r2_{b}")
                nc.vector.reciprocal(r2[:], d2s[:])

            def r1(b):
                # r1[n] = sum_m E1^T[m, n]; free-dim reduce in token-partition
                s = st[b]
                r_s = psmall.tile([P, NN], F32, tag="r_s", name=f"r_s_{b}")
                nc.vector.reduce_sum(r_s[:], s["e12p"][:, :, 0:M],
                                     axis=mybir.AxisListType.X)
                s["r_inv"] = r_inv = psmall.tile([P, NN], F32, tag="r_inv",
                                                 name=f"r_inv_{b}")
                nc.vector.reciprocal(r_inv[:], r_s[:])

            def ex(b):
                # exr = diag(r2) * (E2 @ x)   [16, C]
                s = st[b]
                pse = ps_se.tile([M, C], F32, tag="se", name=f"se_{b}")
                for n in range(NN):
                    nc.tensor.matmul(
                        pse[:], s["e12p"][:, n, 2 * M:3 * M], s["xn"][:, n, :],
                        start=(n == 0), stop=(n == NN - 1))
                s["exr"] = exr = psmall.tile([M, C], BF16, tag="exr",
                                             name=f"exr_{b}")
                nc.scalar.activation(
                    exr[:], pse[:], mybir.ActivationFunctionType.Copy,
                    scale=s["r2"][:])
                # exrT[p, c, m] = exr[m, c*128+p] via XBAR (keeps PE free)
                s["exrT"] = exrT_ = psmall.tile([P, ND, M], BF16, tag="exrT",
                                                name=f"exrT_{b}")
                nc.sync.dma_start_transpose(exrT_[:], exr[:])

            def afw(b):
                # afw = exr @ Wvo^T + b'   [16, C]
                s = st[b]
                psa = ps_se.tile([M, C], F32, tag="se", name=f"afw_{b}")
                for c in range(ND):
                    nc.tensor.matmul(
                        psa[:], s["exrT"][:, c, :], wvo_s[:, c, :],
                        start=(c == 0), stop=(c == ND - 1))
                s["afw"] = afw_ = psmall.tile([M, C], BF16, tag="afw",
                                              name=f"afw_{b}")
                nc.vector.tensor_add(afw_[:], psa[:], bp_s[:])

            def outp(b, rng=None):
                # out chunk = r1 * (E1 @ afw) + b'
                s = st[b]
                if rng is None:
                    rng = range(NN)
                if "o" not in s:
                    s["o"] = pout.tile([P, NN, C], BF16, tag="o", name=f"o_{b}")
                o_s = s["o"]
                for n in rng:
                    po = ps_out.tile([P, C], F32, tag="mm", name=f"mm_{b}")
                    nc.tensor.matmul(
                        po[:], s["e12t"][0:M, ts(n, P)], s["afw"][:],
                        start=True, stop=True)
                    if n % 2 == 0:
                        nc.scalar.activation(
                            o_s[:, n, :], po[:],
                            mybir.ActivationFunctionType.Copy,
                            scale=s["r_inv"][:, n:n + 1])
                    else:
                        nc.vector.tensor_scalar_mul(
                            o_s[:, n, :], po[:], s["r_inv"][:, n:n + 1])

            def store(b, half=None):
                if half is None:
                    nc.gpsimd.dma_start(out_d.ap()[b], st[b]["o"][:])
                else:
                    h = NN // 2
                    sl = slice(half * h, (half + 1) * h)
                    nc.gpsimd.dma_start(out_d.ap()[b][:, sl], st[b]["o"][:, sl])

            # prologue: big loads on the gpsimd (SWDGE) ring, need-ordered
            load_xt(0)
            load_xn(0)
            load_xt(1)
            nc.gpsimd.dma_start(wvo_s[:], wvo_d.ap())
            load_xn(1)

            # software pipeline: round r = loads(r+2), l12(r), afw(r-2),
            # ex(r-1), outp(r-2) -- every PE group's inputs are produced a
            # full round earlier, so the PE never waits mid-round.
            for r in range(B_LOC + 2):
                if r + 2 < B_LOC:
                    load_xt(r + 2)
                    load_xn(r + 2)
                if r < B_LOC:
                    l12(r)
                if 0 <= r - 2:
                    afw(r - 2)
                if r < B_LOC:
                    r2c(r)
                if 0 <= r - 1 < B_LOC:
                    ex(r - 1)
                if r < B_LOC:
                    r1(r)
                if 0 <= r - 2:
                    b_o = r - 2
                    if b_o < B_LOC - 2:
                        outp(b_o)
                        store(b_o)
                    else:
                        # tail: split for earlier store start
                        outp(b_o, range(0, NN // 2))
                        store(b_o, 0)
                        outp(b_o, range(NN // 2, NN))
                        store(b_o, 1)

    nc.compile()
    return nc


def _prep_inputs(x, Wq, bq, Wk, bk, Wv, bv, Wo, bo):
    bf = ml_dtypes.bfloat16
    x32 = np.asarray(x, np.float32)
    # xt[b, p, kk, i, n] = x[b, n, kk*256+i*128+p]  (fp8 DoubleRow planes)
    f8 = ml_dtypes.float8_e4m3
    xt = np.ascontiguousarray(
        x32.transpose(0, 2, 1).reshape(B, 2, 2, P, N).transpose(0, 3, 1, 2, 4)
    ).astype(f8)
    # xn[b, p, o, c] = x[b, o*128+p, c]
    xn = np.ascontiguousarray(
        x32.reshape(B, NN, P, C).transpose(0, 2, 1, 3)).astype(bf)
    # pooled sums, all local batches stacked
    xsum = x32.reshape(B, M, G, C).sum(axis=2)  # [B, M, C]
    Wo64 = np.asarray(Wo, np.float64)
    Wv64 = np.asarray(Wv, np.float64)

    def wtile(w):  # [C, C] -> [P, ND, C] with w[p, o, d] = W[o*128+p, d]
        return np.ascontiguousarray(
            np.asarray(w, np.float32).reshape(ND, P, C).transpose(1, 0, 2)
        ).astype(bf)

    shared = {
        "wvoT": wtile((Wo64 @ Wv64).T.astype(np.float32)),
    }
    bprime = np.asarray(bo, np.float64) + Wo64 @ np.asarray(bv, np.float64)
    shared["bp"] = np.tile(bprime.astype(np.float32), (M, 1)).astype(np.float32)
    in_maps = []
    for ci in range(N_CORES):
        m = dict(shared)
        m["xt"] = np.ascontiguousarray(xt[ci * B_LOC:(ci + 1) * B_LOC])
        m["xn"] = np.ascontiguousarray(xn[ci * B_LOC:(ci + 1) * B_LOC])
        xs_c = xsum[ci * B_LOC:(ci + 1) * B_LOC]  # [B_LOC, M, C]
        # agent products on host (f64): Asum, AWQ^T, AWK^T, c1
        asum64 = xs_c.astype(np.float64) @ np.asarray(Wq, np.float64).T \
            + 64.0 * np.asarray(bq, np.float64)          # [B_LOC, M, C]
        awq = np.einsum('bmd,dc->cbm', asum64, np.asarray(Wq, np.float64))
        awk = np.einsum('bmd,dc->cbm', asum64, np.asarray(Wk, np.float64))
        awqk = np.zeros((C, B_LOC, 3 * M), np.float32)   # [c, b, j]
        awqk[:, :, 0:M] = awq          # E1 rows 0:16
        awqk[:, :, 2 * M:3 * M] = awk  # E2 rows 32:48
        m["awqk"] = np.ascontiguousarray(
            awqk.reshape(2, 2, P, B_LOC, 3 * M).transpose(2, 0, 1, 3, 4)
        ).astype(f8)
        c1 = SCALE * (asum64 @ np.asarray(bq, np.float64))  # [B_LOC, M]
        c12b = np.zeros((3 * M, B_LOC), np.float32)
        c12b[0:M, :] = c1.T.astype(np.float32)
        m["c12b"] = c12b
        in_maps.append(m)
    return in_maps


def _unpermute_out(res):
    # out_d[b, p, o, c] = out[b, o*128+p, c]
    outs = []
    for ci in range(N_CORES):
        o = np.asarray(res.results[ci]["out"], np.float32)  # [B_LOC, P, NN, C]
        outs.append(o.transpose(0, 2, 1, 3).reshape(B_LOC, N, C))
    return np.concatenate(outs, axis=0)


def kernel(x, Wq, bq, Wk, bk, Wv, bv, Wo, bo):
    global _LAST_RESULTS
    nc = _build_program()
    in_maps = _prep_inputs(x, Wq, bq, Wk, bk, Wv, bv, Wo, bo)
    res = run_bass_kernel_spmd(nc, in_maps, list(range(N_CORES)), **_RUN_KWARGS)
    _LAST_RESULTS = res
    return _unpermute_out(res)
